# revision 14
# baseline (speedup 1.0000x reference)
"""Causal multi-head attention block (B=2, T=2048, C=1024, H=16) on 8 TRN2
NeuronCores.

Sharding (Megatron-style): core = (b, hg) with b in {0,1} data-parallel over
batch and hg in {0..3} tensor-parallel over head groups (4 heads each).
Each core computes qkv for its 768 attn_w columns, attention for its 4 heads,
and a partial output projection; the host sums the 4 partials per batch.

Numerics: matmuls run in bf16 (inputs rounded host-side; PSUM accumulates
fp32); softmax is the unstable variant (logits are O(10), exp cannot
overflow); the k-bias is dropped (softmax row-shift invariant) and the v-bias
is constant-folded into an effective output-projection bias on the host.

Attention layout: S^T = K Q^T is computed per head-pair with the two heads
row-packed in the PE array (contraction dim 64 each); softmax reductions then
live on the free axis via a ones-column appended to V (PV yields out^T with
the denominators in row 64).

Engine placement: matmuls on PE; exp on the scalar engine (bf16 out); causal
diagonal masking via a precomputed [128,128] upper-triangular mask multiplied
on the vector engine; softmax normalization = approx-reciprocal (custom DVE)
+ partition_broadcast (the only runtime gpsimd op, so no gpsimd library
swaps) + vector multiply; qkv-projection PSUM->SBUF copies on the scalar
engine (bias fused); v/y copies on vector.
"""

import numpy as np

B, T, C = 2, 2048, 1024
H, HD = 16, 64
P = 128
TT = T // P      # 16 row tiles
NI = T // 512    # 4 query blocks of 512
CT = C // P      # 8 contraction tiles
SCALE = HD ** -0.5

_NC_CACHE = {}


def _build_nc(repeats=1, loop_n=0, parts=("proj", "attn", "y")):
    import concourse.tile as tile
    from concourse import bacc, mybir
    from concourse.bass import ds, ts

    f32 = mybir.dt.float32
    bf16 = mybir.dt.bfloat16
    AF = mybir.ActivationFunctionType

    nc = bacc.Bacc("TRN2", target_bir_lowering=False, debug=False)

    xT_d = nc.declare_dram_parameter("xT", [C, T], bf16, isOutput=False)
    wqk_d = nc.declare_dram_parameter("wqk", [C, 512], bf16, isOutput=False)
    wv_d = nc.declare_dram_parameter("wv", [C, 256], bf16, isOutput=False)
    bq_d = nc.declare_dram_parameter("bq", [256], f32, isOutput=False)
    pw_d = nc.declare_dram_parameter("pw", [256, C], bf16, isOutput=False)
    y_d = nc.declare_dram_parameter("y", [T, C], f32, isOutput=True)

    with (
        tile.TileContext(nc) as tc,
        tc.tile_pool(name="const", bufs=1) as constp,
        tc.tile_pool(name="xw", bufs=1) as xwp,
        tc.tile_pool(name="acts", bufs=1) as actsp,
        tc.tile_pool(name="pt", bufs=4) as ptp,
        tc.tile_pool(name="small", bufs=2) as smallp,
        tc.tile_pool(name="rbp", bufs=2) as rbpool,
        tc.tile_pool(name="ysb", bufs=4) as ysbp,
        tc.tile_pool(name="qkps", bufs=2, space="PSUM") as qkps,
        tc.tile_pool(name="pvps", bufs=2, space="PSUM") as pvps,
    ):
        # ---------- small constants first ----------
        bq_sb = constp.tile([P, 2], f32)
        nc.sync.dma_start(bq_sb[:], bq_d.rearrange("(o p) -> p o", p=P))

        # causal mask for diagonal 128x128 blocks: mask[p, s, x] = (x >= p)
        mask_sb = constp.tile([P, 2, P], bf16)
        nc.vector.memset(mask_sb[:], 1.0)
        for s2 in range(2):
            nc.gpsimd.affine_select(
                out=mask_sb[:, s2, :],
                in_=mask_sb[:, s2, :],
                compare_op=mybir.AluOpType.is_ge,
                fill=0.0,
                base=0,
                pattern=[[1, P]],
                channel_multiplier=-1,
            )

        # ---------- big loads, few triggers, ordered by first use ----------
        xT_sb = xwp.tile([P, CT, T], bf16)
        xTr = xT_d.rearrange("(ct p) t -> p ct t", p=P)
        wqk_sb = xwp.tile([P, CT, 512], bf16)
        wqkr = wqk_d.rearrange("(ct p) j -> p ct j", p=P)
        wv_sb = xwp.tile([P, CT, 256], bf16)
        wvr = wv_d.rearrange("(ct p) j -> p ct j", p=P)
        for ch in range(4):
            nc.sync.dma_start(
                wqk_sb[:, 2 * ch : 2 * ch + 2, :], wqkr[:, 2 * ch : 2 * ch + 2, :]
            )
        for ch in range(4):
            nc.sync.dma_start(
                xT_sb[:, 2 * ch : 2 * ch + 2, 0:1024],
                xTr[:, 2 * ch : 2 * ch + 2, 0:1024],
            )
        nc.sync.dma_start(wv_sb[:, 0:4, :], wvr[:, 0:4, :])
        nc.sync.dma_start(wv_sb[:, 4:8, :], wvr[:, 4:8, :])
        for ch in range(4):
            nc.sync.dma_start(
                xT_sb[:, 2 * ch : 2 * ch + 2, 1024:2048],
                xTr[:, 2 * ch : 2 * ch + 2, 1024:2048],
            )
        pw_sb = constp.tile([P, 2, C], bf16)
        nc.sync.dma_start(pw_sb[:], pw_d.rearrange("(k p) n -> p k n", p=P))

        import contextlib

        _loop_cm = tc.For_i(0, loop_n, 1) if loop_n else contextlib.nullcontext()
        with _loop_cm:
            for _rep in range(repeats):
                # ---------- qkv^T projection ----------
                # qkT layout: [128, 4, T]; jt 0,1 = k^T head-pairs 0,1; jt 2,3 = q^T.
                # Within a jt tile, partitions 0-63 = even head of the pair, 64-127 odd.
                qkT = actsp.tile([P, 4, T], bf16, tag="qkT", name=f"qkT{_rep}")
                v_all = actsp.tile([P, TT, 4, 65], bf16, tag="v_all", name=f"v_all{_rep}")
                att = actsp.tile([P, 2, T], bf16, tag="att", name=f"att{_rep}")
                # ones column for the softmax-denominator rows (d=64 of v_all)
                nc.vector.memset(v_all[:, :, :, 64:65], 1.0)

                def emit_qk_proj(jt, tp):
                    ps = qkps.tile([P, 2, 512], f32, tag="qk", name=f"qkp{_rep}{jt}{tp}")
                    for c in range(CT):
                        for s in range(2):
                            nc.tensor.matmul(
                                ps[:, s, :],
                                wqk_sb[:, c, ts(jt, P)],
                                xT_sb[:, c, ds(1024 * tp + 512 * s, 512)],
                                start=(c == 0),
                                stop=(c == CT - 1),
                            )
                    out = qkT[:, jt, ds(1024 * tp, 1024)].rearrange(
                        "p (s x) -> p s x", s=2
                    )
                    if jt >= 2:
                        nc.vector.tensor_scalar(
                            out,
                            ps[:],
                            scalar1=bq_sb[:, jt - 2 : jt - 1],
                            scalar2=None,
                            op0=mybir.AluOpType.add,
                        )
                    else:
                        nc.vector.tensor_copy(out, ps[:])

                def emit_v_proj(tt):
                    # v_all[p, tt, l, d]: t = 128*tt + p, head l, d 0-63; d=64 is ones.
                    psv = qkps.tile([P, 2, 512], f32, tag="qk", name=f"vp{_rep}{tt}")
                    for c in range(CT):
                        nc.tensor.matmul(
                            psv[:, 0, 0:256],
                            xT_sb[:, c, ts(tt, P)],
                            wv_sb[:, c, :],
                            start=(c == 0),
                            stop=(c == CT - 1),
                        )
                    nc.vector.tensor_copy(
                        v_all[:, tt, :, 0:64],
                        psv[:, 0, 0:256].rearrange("p (l d) -> p l d", l=4),
                    )

                # ---------- output projection (psum slots shared with S tiles) ----------
                def emit_y(tt):
                    psy = qkps.tile([P, 2, 512], f32, tag="qk", name=f"y{_rep}{tt}")
                    y_sb = ysbp.tile([P, 2, 512], f32, tag="ysb", name=f"ys{_rep}{tt}")
                    for n in range(2):
                        for k in range(2):
                            nc.tensor.matmul(
                                psy[:, n, :],
                                att[:, k, ts(tt, P)],
                                pw_sb[:, k, ds(512 * n, 512)],
                                start=(k == 0),
                                stop=(k == 1),
                            )
                        nc.vector.tensor_copy(y_sb[:, n, :], psy[:, n, :])
                    nc.sync.dma_start(
                        y_d[ts(tt, P), :],
                        y_sb[:].rearrange("p n x -> p (n x)"),
                    )

                # ---------- attention ----------
                # S^T tiles: [t_k partitions, t_q free]; one exp per j0 covers both
                # heads of the pair; PV contracts j=t_k with v as lhsT, producing
                # out^T [65, t_q] per head (row 64 = softmax denominators).
                def qk_exp_step(hp, i0, j0):
                    kT = qkT[:, hp, :]
                    qT = qkT[:, 2 + hp, :]
                    c0 = P * j0 - 512 * i0
                    c0p = max(0, c0)
                    w = 512 - c0p
                    psS = qkps.tile(
                        [P, 2, 512], f32, tag="qk", name=f"s{_rep}{hp}{i0}{j0}"
                    )
                    for h01 in range(2):
                        nc.tensor.matmul(
                            psS[:, h01, ds(c0p, w)],
                            kT[64 * h01 : 64 * h01 + 64, ts(j0, P)],
                            qT[64 * h01 : 64 * h01 + 64, ds(512 * i0 + c0p, w)],
                            start=True,
                            stop=True,
                        )
                    pt = ptp.tile(
                        [P, 2, 512], bf16, tag="pt", name=f"pt{_rep}{hp}{i0}{j0}"
                    )
                    nc.scalar.activation(
                        pt[:, :, ds(c0p, w)],
                        psS[:, :, ds(c0p, w)],
                        AF.Exp,
                        scale=SCALE,
                    )
                    if c0 >= 0 and "noselect" not in parts:
                        # zero the below-diagonal triangle of the 128-wide block
                        nc.vector.tensor_mul(
                            pt[:, :, ds(c0, P)],
                            pt[:, :, ds(c0, P)],
                            mask_sb[:],
                        )
                    return pt

                def pv_step(hp, i0, j0, nj, accs, pt):
                    cp = max(0, P * j0 - 512 * i0)
                    wp = 512 - cp
                    for h01 in range(2):
                        nc.tensor.matmul(
                            accs[0:65, h01, ds(cp, wp)],
                            v_all[:, j0, 2 * hp + h01, :],
                            pt[:, h01, ds(cp, wp)],
                            start=(j0 == 0),
                            stop=(j0 == nj - 1),
                        )

                def normalize_step(hp, i0, accs):
                    # accs[0:65, h01, :]: rows 0-63 = out^T (unnormalized),
                    # row 64 = softmax denominator.
                    # 1/d = exp(-ln(d)) on the scalar engine: ln and exp share
                    # an activation table set, so no table reloads.
                    lnv = smallp.tile(
                        [1, 2, 512], f32, tag="ln", name=f"ln{_rep}{hp}{i0}"
                    )
                    nc.scalar.activation(lnv[:], accs[64:65, :, :], AF.Ln)
                    rec = smallp.tile(
                        [1, 2, 512], f32, tag="rec", name=f"rc{_rep}{hp}{i0}"
                    )
                    nc.scalar.activation(rec[:], lnv[:], AF.Exp, scale=-1.0)
                    for h01 in range(2):
                        rb = rbpool.tile(
                            [64, 512], f32, tag="rb", name=f"rb{_rep}{hp}{i0}{h01}"
                        )
                        nc.gpsimd.partition_broadcast(rb[:], rec[0:1, h01, :])
                        nc.vector.tensor_mul(
                            att[64 * h01 : 64 * h01 + 64, hp, ds(512 * i0, 512)],
                            accs[0:64, h01, :],
                            rb[:],
                        )

                # ---------- schedule: attention pipeline with filler work ----------
                prog = []

                def G(hp, i0, mid=None):
                    nj = 4 * i0 + 4
                    for j0 in range(nj):
                        prog.append(("s", hp, i0, j0, nj))
                        if mid and j0 in mid:
                            for fn, a in mid[j0]:
                                prog.append(("f", fn, a))

                def F(fn, *a):
                    prog.append(("f", fn, a))

                def mk_mid(*pairs):
                    # pairs of (step_index, fn, args)
                    m = {}
                    for step, fn, a in pairs:
                        m.setdefault(step, []).append((fn, a))
                    return m

                Y = "y" in parts

                if "attn" in parts:
                    F(emit_qk_proj, 0, 0)
                    F(emit_qk_proj, 2, 0)
                    for tt in range(4):
                        F(emit_v_proj, tt)
                    G(0, 0)
                    F(emit_qk_proj, 1, 0)
                    F(emit_qk_proj, 3, 0)
                    G(1, 0)
                    F(emit_qk_proj, 0, 1)
                    F(emit_qk_proj, 2, 1)
                    # g(0,1): v4-7 early, y(0) rows after normalize(1,0) lands
                    G(0, 1, mid=mk_mid(
                        (0, emit_v_proj, (4,)), (1, emit_v_proj, (5,)),
                        (2, emit_v_proj, (6,)), (3, emit_v_proj, (7,)),
                        *([(4, emit_y, (0,)), (6, emit_y, (1,))] if Y else []),
                    ))
                    G(1, 1, mid=mk_mid(
                        *([(1, emit_y, (2,)), (3, emit_y, (3,))] if Y else []),
                    ))
                    F(emit_qk_proj, 1, 1)
                    F(emit_qk_proj, 3, 1)
                    G(0, 2, mid=mk_mid(
                        (0, emit_v_proj, (8,)), (1, emit_v_proj, (9,)),
                        (2, emit_v_proj, (10,)), (3, emit_v_proj, (11,)),
                        *([(5, emit_y, (4,)), (7, emit_y, (5,))] if Y else []),
                    ))
                    G(1, 2, mid=mk_mid(
                        *([(1, emit_y, (6,)), (3, emit_y, (7,))] if Y else []),
                    ))
                    G(0, 3, mid=mk_mid(
                        (0, emit_v_proj, (12,)), (1, emit_v_proj, (13,)),
                        (2, emit_v_proj, (14,)), (3, emit_v_proj, (15,)),
                        *([(5, emit_y, (8,)), (7, emit_y, (9,))] if Y else []),
                    ))
                    G(1, 3, mid=mk_mid(
                        *([(1, emit_y, (10,)), (3, emit_y, (11,))] if Y else []),
                    ))

                    LAG = 3
                    pend = []  # (hp, i0, j0, nj, pt)
                    accs_map = {}

                    def pop_one():
                        hp, i0, j0, nj, pt = pend.pop(0)
                        if "nopv" in parts:
                            return
                        pv_step(hp, i0, j0, nj, accs_map[(hp, i0)], pt)
                        if j0 == nj - 1:
                            if "nonorm" not in parts:
                                normalize_step(hp, i0, accs_map[(hp, i0)])
                            del accs_map[(hp, i0)]

                    for item in prog:
                        if item[0] == "f":
                            _, fn, a = item
                            fn(*a)
                            continue
                        _, hp, i0, j0, nj = item
                        if j0 == 0:
                            accs_map[(hp, i0)] = pvps.tile(
                                [P, 2, 512], f32, tag="pv",
                                name=f"acc{_rep}{hp}{i0}",
                            )
                        pt = qk_exp_step(hp, i0, j0)
                        pend.append((hp, i0, j0, nj, pt))
                        if len(pend) > LAG:
                            pop_one()
                    while pend:
                        pop_one()
                    if "y" in parts:
                        for tt in range(12, 16):
                            emit_y(tt)
                elif "proj" in parts:
                    for tp in range(2):
                        emit_qk_proj(0, tp)
                        emit_qk_proj(2, tp)
                    for tt in range(TT):
                        emit_v_proj(tt)
                    for tp in range(2):
                        emit_qk_proj(1, tp)
                        emit_qk_proj(3, tp)

    nc.compile()

    # Every activation func used (Exp, Ln, Identity, Copy) lives in the
    # natural_log_exp_and_others table set, but the table-load pass alternates
    # between the exp-only and ln-only sets, inserting a 1.3us ACT_TABLE_LOAD
    # around every softmax-normalize.  Point the first load at the combined
    # set and drop the redundant ones (they carry no sync_info).
    from concourse.hw_specs import get_activation_tables

    combined_id = list(get_activation_tables(nc.m.arch)).index(
        "natural_log_exp_and_others"
    )
    first_load = True
    for blk in nc.main_func.blocks:
        keep = []
        for inst in blk.instructions:
            if type(inst).__name__ == "InstLoadActFuncSet":
                assert inst.sync_info is None
                if first_load:
                    inst.act_func_set_id = combined_id
                    first_load = False
                else:
                    continue
            keep.append(inst)
        if len(keep) != len(blk.instructions):
            blk.instructions[:] = keep
    return nc


def _get_nc(repeats=1, loop_n=0, parts=("proj", "attn", "y")):
    key = ("nc", repeats, loop_n, parts)
    if key not in _NC_CACHE:
        _NC_CACHE[key] = _build_nc(repeats, loop_n, parts)
    return _NC_CACHE[key]


def _make_in_maps(x, attn_w, attn_b, proj_w, proj_b):
    import ml_dtypes

    bf16 = ml_dtypes.bfloat16
    _make_in_maps.beff = {}
    in_maps = []
    for core in range(8):
        b, hg = core // 4, core % 4
        cs = 256 * hg
        k_cols = attn_w[:, cs : cs + 256]
        q_cols = attn_w[:, 1024 + cs : 1024 + cs + 256]
        v_cols = attn_w[:, 2048 + cs : 2048 + cs + 256]
        b_q = attn_b[1024 + cs : 1024 + cs + 256]
        b_v = attn_b[2048 + cs : 2048 + cs + 256]
        pw = proj_w[cs : cs + 256, :]
        beff = (b_v.astype(np.float64) @ pw.astype(np.float64)).astype(np.float32)
        if hg == 0:
            beff = beff + proj_b
        _make_in_maps.beff[core] = beff
        in_maps.append(
            {
                "xT": np.ascontiguousarray(x[b].T).astype(bf16),
                "wqk": np.ascontiguousarray(
                    np.concatenate([k_cols, q_cols], axis=1)
                ).astype(bf16),
                "wv": np.ascontiguousarray(v_cols).astype(bf16),
                "bq": np.ascontiguousarray(b_q),
                "pw": np.ascontiguousarray(pw).astype(bf16),
            }
        )
    return in_maps


def kernel(x, attn_w, attn_b, proj_w, proj_b, _spmd_kwargs=None):
    from concourse.bass_utils import run_bass_kernel_spmd

    x = np.asarray(x, dtype=np.float32)
    attn_w = np.asarray(attn_w, dtype=np.float32)
    attn_b = np.asarray(attn_b, dtype=np.float32)
    proj_w = np.asarray(proj_w, dtype=np.float32)
    proj_b = np.asarray(proj_b, dtype=np.float32)

    nc = _get_nc((_spmd_kwargs or {}).pop("repeats", 1) if _spmd_kwargs else 1)
    in_maps = _make_in_maps(x, attn_w, attn_b, proj_w, proj_b)
    res = run_bass_kernel_spmd(
        nc, in_maps, core_ids=list(range(8)), **(_spmd_kwargs or {})
    )
    out = np.zeros((B, T, C), dtype=np.float32)
    for core in range(8):
        out[core // 4] += res.results[core]["y"]
    for core in range(8):
        out[core // 4] += _make_in_maps.beff[core][None, :]
    if _spmd_kwargs:
        kernel.last_results = res
    return out
